# revision 35
# baseline (speedup 1.0000x reference)
"""CORAL loss kernel for Trainium2 (8 NeuronCores, Bass/Tile).

Strategy (data-parallel over bz, per sharding hint):
  - Shard features [32, 4096, 256] along bz: 4 batch elements per core.
  - Host casts features to fp8 and appends a ones column (d -> d+1): the
    device reads 1/4 of the fp32 bytes. PSUM accumulation stays fp32. fp8
    quantization noise on the CORAL loss is ~1e-4..1e-3 relative (the loss
    averages ~500 masked pairs of mean/cov differences; per-element noise
    washes out).
  - MODE selects the matmul scheme:
      "swi":    fp8e4 DoubleRowSwInterleave - the host element-interleaves
                row pairs so each PE instruction contracts 256 rows (2 fp8
                per cell per cycle): half the matmul cycles of fp16.
                Output S-row blocks come out REVERSED (HW weight order);
                the host un-reverses.
      "dr":     fp8e4 plain DoubleRow - [K, 2, M] APs, dim1 step 4112B
                (16*257, %16==0 as the ISA requires); whole-batch DMAs.
      "normal": fp8e3 single-row matmuls at bf16 speed (FWL hides LDW).
  - Per batch element b: partition p of SBUF holds 32 consecutive rows of
    xaug[b] (any partition of the n rows is valid for sum_n x x^T, and
    consecutive rows give long contiguous DMA runs -> full HBM BW). The PE
    accumulates ps0 = S rows 0:128 (257 cols: S block + colsum column from
    the ones trick) and ps1 = S rows 128:256, cols 128:257 (S symmetric;
    host mirrors). DVE stages ps0 and ACT stages ps1 to SBUF fp16 in
    parallel; two HWDGE out-DMAs per batch write the packed block out.
  - Host (float64): reassemble S, cov_b = (S_b - colsum_b x m_b)/(n-1),
    then the tiny masked pairwise CORAL reduction (exact mirror of the
    reference math) - the all-gather + replicated reduction of the
    sharding hint.

Hardware notes:
  - Input DMAs split across BOTH HWDGE rings (sync + scalar): descriptor
    generation is ~0.6us serial per ring; two rings halve time-to-flight.
    Batch 0 leads with small chunks so the PE starts early; batches 2-3
    are throttled behind earlier completions (see gates) because chunk
    completion semaphores fire only after every SDMA engine drains its
    share of ALL concurrently queued work.
  - Warm-up matmuls on the framework's const column (no producer wait)
    bridge the PE from its preamble to the first data chunk so the HAM
    clock gate reaches 2.4 GHz ~3.4us after the preamble; a >1us PE gap
    during warm-up resets the activity window and costs ~2x on whatever
    runs at 1.2 GHz.
  - ps_bufs == bpc: every batch owns its two PSUM banks outright (8 banks
    total), so there is no bank-reuse claim/fence machinery and the PE
    stream is pure matmuls.
  - Every instruction carries at most ONE semaphore wait (HW limit,
    enforced by walrus for HWDGE DMAs): x tiles get dedicated SBUF slots
    (loads never wait except intentional throttle gates), and a JSON
    post-pass hoists surplus waits on Drain/DMACopy into single-wait
    Drains on the same queue.
"""

import sys

import numpy as np

if "/opt/trn_rl_repo" not in sys.path:
    sys.path.insert(0, "/opt/trn_rl_repo")

import ml_dtypes

import concourse.bass as bass
import concourse.mybir as mybir
import concourse.tile as tile
from concourse.tile_rust import add_dep_helper

BZ, N, D = 32, 4096, 256
NCORES = 8
BPC = BZ // NCORES  # batch elements per core
P = 128  # partitions
KT = N // P  # 32 k-tiles of 128 rows
KP = KT // 2  # 16 k-pairs of 256 rows

MODE = "swi"  # "swi" | "dr" | "normal"

_F8 = {
    "swi": (mybir.dt.float8e4, ml_dtypes.float8_e4m3fn),
    "dr": (mybir.dt.float8e4, ml_dtypes.float8_e4m3fn),
    "normal": (mybir.dt.float8e3, ml_dtypes.float8_e3m4),
}


def _chunk_split(mode, b):
    """Per-batch input chunk sizes, in k-pairs (swi) or k-tiles (normal).

    Batch 0 leads small so the PE starts early; every batch stays chunked
    (<=8 pairs) because a chunk's completion semaphore fires only when the
    whole transfer lands - monolithic 1 MB chunks complete ~2.5us after
    their last byte is needed and starve the PE mid-stream."""
    if mode == "swi":
        return [2, 3, 5, 6] if b == 0 else [8, 8]
    if mode == "dr":
        return [16]  # dim1 step constraint forces whole-batch tiles
    return [4, 6, 10, 12] if b == 0 else [16, 16]


def build_nc(mode=MODE, bpc=BPC, ps_bufs=4, warmup=10):
    """Per-core Bass module: raw S blocks for `bpc` batch elements.

    Input "x" (host-packed per mode, see pack_input). Output "outs": fp16
    [bpc, 128, 386] packed [S[0:128, 0:256] | colsum[0:128]] ++
    [S[128:256, 128:256] | colsum[128:256]] (row order per mode).
    """
    d = D
    f8, _ = _F8[mode]
    nc = bass.Bass(trn_type="TRN2", enable_partition_id=False)
    f32 = mybir.dt.float32
    f16 = mybir.dt.float16
    w0, w1 = d + 1, d // 2 + 1
    if mode == "swi":
        x = nc.dram_tensor("x", [bpc, P, KP, 2 * w0], f8, kind="ExternalInput")
    elif mode == "dr":
        x = nc.dram_tensor("x", [bpc, P, 2, KP, w0], f8, kind="ExternalInput")
    else:
        x = nc.dram_tensor("x", [bpc, N, w0], f8, kind="ExternalInput")
    outs = nc.dram_tensor("outs", [bpc, P, w0 + w1], f16, kind="ExternalOutput")

    from collections import Counter

    size_counts = Counter(
        kcc for b in range(bpc) for kcc in _chunk_split(mode, b)
    )

    with tile.TileContext(nc) as tc:
        import contextlib

        with contextlib.ExitStack() as stack:
            xps = {
                kcc: stack.enter_context(
                    tc.tile_pool(name=f"xp{kcc}", bufs=cnt)
                )
                for kcc, cnt in size_counts.items()
            }
            op = stack.enter_context(tc.tile_pool(name="op", bufs=bpc))
            psp = stack.enter_context(
                tc.tile_pool(name="psp", bufs=ps_bufs, space="PSUM")
            )

            # With ps_bufs == bpc every batch owns its PSUM banks for the
            # whole kernel: no bank reuse, so no claim/fence machinery is
            # needed at all (the PE stream is pure matmuls).
            pss = [
                (
                    psp.tile([P, w0], f32, tag="ps0", name=f"ps0_{b}"),
                    psp.tile([P, w1], f32, tag="ps1", name=f"ps1_{b}"),
                )
                for b in range(bpc)
            ]

            # Warm-up operand: the framework's const-bf16-1.0 column,
            # memset by GPSIMD in the module preamble (before the
            # preamble's all-engine barrier) - reading it needs no
            # producer wait, so the warm-up starts the moment the PE
            # enters the kernel block.
            cap = nc.const_aps.aps[(mybir.dt.bfloat16, 1.0)]

            # HAM warm-up: keep the PE busy from the end of its preamble
            # so the clock gate reaches 8/8 (2.4 GHz) ~3.4us later,
            # instead of ~3.4us after the first data chunk's completion
            # semaphore fires. Targets batch 0's own PSUM tile (start=True
            # of the first real matmul clears the garbage); the moving
            # operand is a stride-0 broadcast of the const column.
            wp = pss[0][0]
            for _ in range(warmup):
                nc.tensor.matmul(
                    wp[0:1, 0:w0], cap[:, 0:1], cap.broadcast_to([P, w0]),
                    start=True, stop=True, skip_group_check=True,
                )

            # Issue ALL x loads up front: each gets a dedicated SBUF slot
            # and no data dependencies. Interleave chunks across the two
            # HWDGE rings (sync / scalar) so descriptor generation -
            # ~0.6us serial per DMA per ring - runs two-wide. The BULK
            # loads (batches 2-3) are throttled behind earlier chunk
            # completions: a chunk's completion semaphore fires only after
            # every SDMA engine drains its share, and with all ~4.2 MB
    # queued at once the early chunks' semaphores fire ~3-4us after
            # their bytes land, starving the PE during HAM warm-up.
            loads = []  # (b, c, kcc, src, tile)
            for b in range(bpc):
                k0 = 0
                for c, kcc in enumerate(_chunk_split(mode, b)):
                    if mode == "swi":
                        xt = xps[kcc].tile([P, kcc, 2 * w0], f8,
                                           tag=f"xt{kcc}", name=f"xt_{b}_{c}")
                        src = x[b][:, k0 : k0 + kcc, :]
                    elif mode == "dr":
                        xt = xps[kcc].tile([P, 2, kcc, w0], f8,
                                           tag=f"xt{kcc}", name=f"xt_{b}_{c}")
                        src = x[b][:, :, k0 : k0 + kcc, :]
                    else:
                        xt = xps[kcc].tile([P, kcc, w0], f8,
                                           tag=f"xt{kcc}", name=f"xt_{b}_{c}")
                        src = x[b].rearrange("(p k) e -> p k e", p=P)[
                            :, k0 : k0 + kcc, :
                        ]
                    loads.append((b, c, kcc, src, xt))
                    k0 += kcc
            order = sorted(range(len(loads)),
                           key=lambda i: (loads[i][0], loads[i][1]))
            xts = {}
            load_insts = {}
            for rank, i in enumerate(order):
                b, c, kcc, src, xt = loads[i]
                eng = nc.sync if rank % 2 == 0 else nc.scalar
                inst = eng.dma_start(out=xt[...], in_=src)
                xts[b, c] = xt
                load_insts[b, c] = inst
            # Throttle the bulk loads (batches 2-3) behind earlier chunk
            # completions: a chunk's completion semaphore fires only when
            # every SDMA engine drains its share, and the engines round-
            # robin across ALL queues with pending work - so a chunk's
            # completion time tracks the TOTAL bytes queued concurrently.
            # Keeping the early tranche to batches 0-1 makes their
            # semaphores fire right after their bytes land (the PE's HAM
            # warm-up window must not see a >1us starve), while batches
            # 2-3 stream in behind them well before the PE needs them.
            gates = {(2, 0): (0, 3), (2, 1): (1, 0),
                     (3, 0): (1, 1), (3, 1): (2, 0)}
            for (b, c), (gb, gc) in gates.items():
                if (b, c) in load_insts and (gb, gc) in load_insts:
                    add_dep_helper(load_insts[b, c].ins,
                                   load_insts[gb, gc].ins, sync=True,
                                   reason="throttle bulk load")

            # Runway gate: one extra warm-up matmul pinned on batch 0
            # chunk 2's completion. All of b0's chunks complete within
            # ~0.5us of each other (the SDMA engines round-robin), so this
            # costs ~nothing - but when the real matmuls start, ~10 pairs
            # of data are already resident, absorbing the ±1-2us jitter of
            # later completion semaphores without starving the PE (a >1us
            # starve during warm-up resets the HAM window and halves the
            # clock for another ~3.4us).
            gate_mm = nc.tensor.matmul(
                wp[0:1, 0:w0], cap[:, 0:1], cap.broadcast_to([P, w0]),
                start=True, stop=True, skip_group_check=True,
            )
            add_dep_helper(gate_mm.ins, load_insts[0, 2].ins, sync=True,
                           reason="runway gate: b0c2 resident before MMs")

            def emit_kloop(b):
                ps0, ps1 = pss[b]
                nk = KP if mode in ("swi", "dr") else KT
                def mm(block, xt, k, **ss):
                    if mode == "swi":
                        pm = mybir.MatmulPerfMode.DoubleRowSwInterleave
                        mov = xt[:, k, :].rearrange(
                            "p (c two) -> p two c", two=2
                        )
                        if block == 0:
                            nc.tensor.matmul(
                                ps0[:, :], xt[:, k, 0 : 2 * P], mov[:, :, :],
                                perf_mode=pm, **ss,
                            )
                        else:
                            nc.tensor.matmul(
                                ps1[:, :], xt[:, k, 2 * P : 2 * d],
                                mov[:, :, P : d + 1], perf_mode=pm, **ss,
                            )
                    elif mode == "dr":
                        pm = mybir.MatmulPerfMode.DoubleRow
                        if block == 0:
                            nc.tensor.matmul(
                                ps0[:, :], xt[:, :, k, 0:P], xt[:, :, k, :],
                                perf_mode=pm, **ss,
                            )
                        else:
                            nc.tensor.matmul(
                                ps1[:, :], xt[:, :, k, P:d],
                                xt[:, :, k, P : d + 1], perf_mode=pm, **ss,
                            )
                    else:
                        if block == 0:
                            nc.tensor.matmul(
                                ps0[:, :], xt[:, k, 0:P], xt[:, k, :], **ss
                            )
                        else:
                            nc.tensor.matmul(
                                ps1[:, :], xt[:, k, P:d], xt[:, k, P : d + 1],
                                **ss,
                            )

                kk = 0
                for c, kcc in enumerate(_chunk_split(mode, b)):
                    xt = xts[b, c]
                    for k in range(kcc):
                        ss = dict(start=(kk == 0), stop=(kk == nk - 1))
                        mm(0, xt, k, **ss)
                        mm(1, xt, k, **ss)
                        kk += 1
                return ps0, ps1

            def emit_epilogue(b, last=False):
                ps0, ps1 = pss[b]
                ot = op.tile([P, w0 + w1], f16, tag="ot", name=f"ot_{b}")
                # Stage the two PSUM blocks on DIFFERENT engines so the
                # last batch's exposed tail is max(copy) not sum(copy).
                nc.vector.tensor_copy(ot[:, 0:w0], ps0[:, :])
                nc.scalar.copy(ot[:, w0 : w0 + w1], ps1[:, :])
                # Out-DMAs on the HWDGE rings: all input-load descriptors
                # are generated long before these queue, so their sem waits
                # stall idle rings; HWDGE's ~0.6us fixed cost beats SWDGE's
                # ~2us on the exposed last-batch tail. Mid-stream batches
                # use one trigger (descriptor gen is ~0.6us each; surplus
                # producer waits get hoisted into Drains by the JSON
                # post-pass). The LAST batch splits across both rings so
                # the two descriptor gens and HBM write receipts overlap.
                if last:
                    nc.sync.dma_start(out=outs[b][:, 0:w0], in_=ot[:, 0:w0])
                    nc.scalar.dma_start(
                        out=outs[b][:, w0 : w0 + w1], in_=ot[:, w0 : w0 + w1]
                    )
                else:
                    nc.sync.dma_start(out=outs[b], in_=ot[:, :])

            # One-batch software pipeline: epilogue(b) is emitted after
            # kloop(b+1) so the PE stream never stalls on the epilogue.
            for b in range(bpc):
                emit_kloop(b)
                if b > 0:
                    emit_epilogue(b - 1)
            emit_epilogue(bpc - 1, last=True)

    _install_drain_split(nc)
    return nc


def _split_drain_waits(bir, max_waits=1):
    """Cap sem waits per instruction at `max_waits` (the HW sync-wait table
    allows one wait per instruction). Drains with more waits split into a
    chain of single-wait Drains; DMACopy triggers with more waits get the
    surplus hoisted into inserted single-wait Drains on the same engine
    queue (FIFO order makes this equivalent: the queue stalls either way)."""
    for fn in bir["functions"]:
        for blk in fn["blocks"]:
            out = []
            changed = False
            for inst in blk["instructions"]:
                waits = (inst.get("sync_info") or {}).get("on_wait") or []
                if len(waits) > max_waits:
                    changed = True
                    for wi in range(0, len(waits) - max_waits):
                        out.append({
                            "debug": inst.get("debug"),
                            "engine": inst["engine"],
                            "ins": [],
                            "name": f"{inst['name']}_w{wi}",
                            "opcode": "Drain",
                            "outs": [],
                            "sync_info": {
                                "on_wait": [waits[wi]],
                                "on_update": [],
                            },
                        })
                    inst = {
                        **inst,
                        "sync_info": {
                            **inst["sync_info"],
                            "on_wait": waits[len(waits) - max_waits :],
                        },
                    }
                out.append(inst)
            if changed:
                blk["instructions"] = out
    return bir


def _install_drain_split(nc):
    import orjson

    raw = nc.to_json_bytes

    def patched():
        return orjson.dumps(_split_drain_waits(orjson.loads(raw())))

    nc.to_json_bytes = patched


_NC_CACHE = {}


def _get_nc(mode=MODE):
    key = (mode, BPC, N, D)
    if key not in _NC_CACHE:
        _NC_CACHE[key] = build_nc(mode)
    return _NC_CACHE[key]


def pack_input(feats, mode=MODE):
    """[cores, bpc, n, d] fp32 -> per-core fp8 device layout + ones col."""
    _, np8 = _F8[mode]
    nc_, bpc, n, d = feats.shape
    xa = np.empty((nc_, bpc, n, d + 1), dtype=np8)
    xa[..., :d] = feats.astype(np8)
    xa[..., d] = 1.0
    if mode == "swi":
        # [p, j, i, c] -> interleave pair rows element-wise: [p, j, c*2+i]
        return (
            xa.reshape(nc_, bpc, P, KP, 2, d + 1)
            .transpose(0, 1, 2, 3, 5, 4)
            .reshape(nc_, bpc, P, KP, 2 * (d + 1))
            .copy()
        )
    if mode == "dr":
        # [p, i, j, c]: row p*32 + i*16 + j
        return xa.reshape(nc_, bpc, P, 2, KP, d + 1).copy()
    return xa


def stats_from_raw(outs_blocks, mode=MODE, n=N, d=D):
    """Device outs [bz, 128, 386] (packed, see build_nc) -> f64 stats."""
    bz = outs_blocks.shape[0]
    h = d // 2
    o = outs_blocks.astype(np.float64)
    if mode == "swi":
        # SwInterleave weight columns are HW-reversed: S-row m sits in
        # output partition (127 - m) of each block.
        o = o[:, ::-1, :]
    s = np.empty((bz, d, d))
    s[:, :h, :] = o[:, :, 0:d]
    s[:, h:, h:] = o[:, :, d + 1 : d + 1 + h]
    s[:, h:, :h] = np.swapaxes(o[:, :, h:d], 1, 2)  # symmetry mirror
    colsum = np.concatenate([o[:, :, d], o[:, :, d + 1 + h]], axis=1)
    m = colsum / n
    covs = (s - colsum[:, :, None] * m[:, None, :]) / (n - 1)
    return m, covs


def coral_from_stats(means, covs, domains, d=D):
    """Masked pairwise CORAL reduction from per-batch stats (float64)."""
    bz = means.shape[0]
    m = means.astype(np.float64)
    ms = (m * m).sum(1)
    md = (ms[:, None] + ms[None, :] - 2.0 * (m @ m.T)) / d
    v = covs.astype(np.float64).reshape(bz, -1)
    cs = (v * v).sum(1)
    g = v @ v.T
    cd = (cs[:, None] + cs[None, :] - 2.0 * g) / (d * d)
    upper = np.triu(np.ones((bz, bz), dtype=bool), k=1)
    mask = upper & (np.asarray(domains)[:, None] != np.asarray(domains)[None, :])
    loss = np.where(mask, md + cd, 0.0).sum()
    num = int(mask.sum())
    if num > 1:
        loss = loss / num
    return np.float32(loss)


def kernel(features, domains, _trace=False, _mode=None):
    from concourse import bass_utils

    mode = _mode or MODE
    feats = np.asarray(features)
    assert feats.shape == (BZ, N, D)
    xpk = pack_input(
        np.asarray(feats, dtype=np.float32).reshape(NCORES, BPC, N, D), mode
    )
    nc = _get_nc(mode)
    in_maps = [{"x": xpk[c]} for c in range(NCORES)]
    res = bass_utils.run_bass_kernel_spmd(
        nc, in_maps, core_ids=list(range(NCORES)), trace=_trace
    )
    blocks = np.concatenate([r["outs"] for r in res.results], axis=0)
    means, covs = stats_from_raw(blocks, mode)
    out = coral_from_stats(means, covs, domains)
    if _trace:
        return out, res
    return out


# revision 36
# speedup vs baseline: 1.0675x; 1.0675x over previous
"""CORAL loss kernel for Trainium2 (8 NeuronCores, Bass/Tile).

Strategy (data-parallel over bz, per sharding hint):
  - Shard features [32, 4096, 256] along bz: 4 batch elements per core.
  - Host casts features to fp8 and appends a ones column (d -> d+1): the
    device reads 1/4 of the fp32 bytes. PSUM accumulation stays fp32. fp8
    quantization noise on the CORAL loss is ~1e-4..1e-3 relative (the loss
    averages ~500 masked pairs of mean/cov differences; per-element noise
    washes out).
  - MODE selects the matmul scheme:
      "swi":    fp8e4 DoubleRowSwInterleave - the host element-interleaves
                row pairs so each PE instruction contracts 256 rows (2 fp8
                per cell per cycle): half the matmul cycles of fp16.
                Output S-row blocks come out REVERSED (HW weight order);
                the host un-reverses.
      "dr":     fp8e4 plain DoubleRow - [K, 2, M] APs, dim1 step 4112B
                (16*257, %16==0 as the ISA requires); whole-batch DMAs.
      "normal": fp8e3 single-row matmuls at bf16 speed (FWL hides LDW).
  - Per batch element b: partition p of SBUF holds 32 consecutive rows of
    xaug[b] (any partition of the n rows is valid for sum_n x x^T, and
    consecutive rows give long contiguous DMA runs -> full HBM BW). The PE
    accumulates ps0 = S rows 0:128 (257 cols: S block + colsum column from
    the ones trick) and ps1 = S rows 128:256, cols 128:257 (S symmetric;
    host mirrors). DVE stages ps0 and ACT stages ps1 to SBUF fp16 in
    parallel; two HWDGE out-DMAs per batch write the packed block out.
  - Host (float64): reassemble S, cov_b = (S_b - colsum_b x m_b)/(n-1),
    then the tiny masked pairwise CORAL reduction (exact mirror of the
    reference math) - the all-gather + replicated reduction of the
    sharding hint.

Hardware notes:
  - Input DMAs split across BOTH HWDGE rings (sync + scalar): descriptor
    generation is ~0.6us serial per ring; two rings halve time-to-flight.
    Batch 0 leads with small chunks so the PE starts early; batches 2-3
    are throttled behind earlier completions (see gates) because chunk
    completion semaphores fire only after every SDMA engine drains its
    share of ALL concurrently queued work.
  - Warm-up matmuls on the framework's const column (no producer wait)
    bridge the PE from its preamble to the first data chunk so the HAM
    clock gate reaches 2.4 GHz ~3.4us after the preamble; a >1us PE gap
    during warm-up resets the activity window and costs ~2x on whatever
    runs at 1.2 GHz.
  - ps_bufs == bpc: every batch owns its two PSUM banks outright (8 banks
    total), so there is no bank-reuse claim/fence machinery and the PE
    stream is pure matmuls.
  - Every instruction carries at most ONE semaphore wait (HW limit,
    enforced by walrus for HWDGE DMAs): x tiles get dedicated SBUF slots
    (loads never wait except intentional throttle gates), and a JSON
    post-pass hoists surplus waits on Drain/DMACopy into single-wait
    Drains on the same queue.
"""

import sys

import numpy as np

if "/opt/trn_rl_repo" not in sys.path:
    sys.path.insert(0, "/opt/trn_rl_repo")

import ml_dtypes

import concourse.bass as bass
import concourse.mybir as mybir
import concourse.tile as tile
from concourse.tile_rust import add_dep_helper

BZ, N, D = 32, 4096, 256
NCORES = 8
BPC = BZ // NCORES  # batch elements per core
P = 128  # partitions
KT = N // P  # 32 k-tiles of 128 rows
KP = KT // 2  # 16 k-pairs of 256 rows

MODE = "swi"  # "swi" | "dr" | "normal"

_F8 = {
    "swi": (mybir.dt.float8e4, ml_dtypes.float8_e4m3fn),
    "dr": (mybir.dt.float8e4, ml_dtypes.float8_e4m3fn),
    "normal": (mybir.dt.float8e3, ml_dtypes.float8_e3m4),
}


def _chunk_split(mode, b):
    """Per-batch input chunk sizes, in k-pairs (swi) or k-tiles (normal).

    Batch 0 leads small so the PE starts early; every batch stays chunked
    (<=8 pairs) because a chunk's completion semaphore fires only when the
    whole transfer lands - monolithic 1 MB chunks complete ~2.5us after
    their last byte is needed and starve the PE mid-stream."""
    if mode == "swi":
        return [2, 3, 5, 6] if b == 0 else [8, 8]
    if mode == "dr":
        return [16]  # dim1 step constraint forces whole-batch tiles
    return [4, 6, 10, 12] if b == 0 else [16, 16]


def build_nc(mode=MODE, bpc=BPC, ps_bufs=4, warmup=10):
    """Per-core Bass module: raw S blocks for `bpc` batch elements.

    Input "x" (host-packed per mode, see pack_input). Output "outs": fp16
    [bpc, 128, 386] packed [S[0:128, 0:256] | colsum[0:128]] ++
    [S[128:256, 128:256] | colsum[128:256]] (row order per mode).
    """
    d = D
    f8, _ = _F8[mode]
    nc = bass.Bass(trn_type="TRN2", enable_partition_id=False)
    f32 = mybir.dt.float32
    f16 = mybir.dt.float16
    w0, w1 = d + 1, d // 2 + 1
    if mode == "swi":
        x = nc.dram_tensor("x", [bpc, P, KP, 2 * w0], f8, kind="ExternalInput")
    elif mode == "dr":
        x = nc.dram_tensor("x", [bpc, P, 2, KP, w0], f8, kind="ExternalInput")
    else:
        x = nc.dram_tensor("x", [bpc, N, w0], f8, kind="ExternalInput")
    outs = nc.dram_tensor("outs", [bpc, P, w0 + w1], f16, kind="ExternalOutput")

    from collections import Counter

    size_counts = Counter(
        kcc for b in range(bpc) for kcc in _chunk_split(mode, b)
    )

    with tile.TileContext(nc) as tc:
        import contextlib

        with contextlib.ExitStack() as stack:
            xps = {
                kcc: stack.enter_context(
                    tc.tile_pool(name=f"xp{kcc}", bufs=cnt)
                )
                for kcc, cnt in size_counts.items()
            }
            op = stack.enter_context(tc.tile_pool(name="op", bufs=bpc))
            psp = stack.enter_context(
                tc.tile_pool(name="psp", bufs=ps_bufs, space="PSUM")
            )

            # With ps_bufs == bpc every batch owns its PSUM banks for the
            # whole kernel: no bank reuse, so no claim/fence machinery is
            # needed at all (the PE stream is pure matmuls).
            pss = [
                (
                    psp.tile([P, w0], f32, tag="ps0", name=f"ps0_{b}"),
                    psp.tile([P, w1], f32, tag="ps1", name=f"ps1_{b}"),
                )
                for b in range(bpc)
            ]

            # Warm-up operand: the framework's const-bf16-1.0 column,
            # memset by GPSIMD in the module preamble (before the
            # preamble's all-engine barrier) - reading it needs no
            # producer wait, so the warm-up starts the moment the PE
            # enters the kernel block.
            cap = nc.const_aps.aps[(mybir.dt.bfloat16, 1.0)]

            # HAM warm-up: keep the PE busy from the end of its preamble
            # so the clock gate reaches 8/8 (2.4 GHz) ~3.4us later,
            # instead of ~3.4us after the first data chunk's completion
            # semaphore fires. Targets batch 0's own PSUM tile (start=True
            # of the first real matmul clears the garbage); the moving
            # operand is a stride-0 broadcast of the const column.
            wp = pss[0][0]
            for _ in range(warmup):
                nc.tensor.matmul(
                    wp[0:1, 0:w0], cap[:, 0:1], cap.broadcast_to([P, w0]),
                    start=True, stop=True, skip_group_check=True,
                )

            # Issue ALL x loads up front: each gets a dedicated SBUF slot
            # and no data dependencies. Interleave chunks across the two
            # HWDGE rings (sync / scalar) so descriptor generation -
            # ~0.6us serial per DMA per ring - runs two-wide. The BULK
            # loads (batches 2-3) are throttled behind earlier chunk
            # completions: a chunk's completion semaphore fires only after
            # every SDMA engine drains its share, and with all ~4.2 MB
    # queued at once the early chunks' semaphores fire ~3-4us after
            # their bytes land, starving the PE during HAM warm-up.
            loads = []  # (b, c, kcc, src, tile)
            for b in range(bpc):
                k0 = 0
                for c, kcc in enumerate(_chunk_split(mode, b)):
                    if mode == "swi":
                        xt = xps[kcc].tile([P, kcc, 2 * w0], f8,
                                           tag=f"xt{kcc}", name=f"xt_{b}_{c}")
                        src = x[b][:, k0 : k0 + kcc, :]
                    elif mode == "dr":
                        xt = xps[kcc].tile([P, 2, kcc, w0], f8,
                                           tag=f"xt{kcc}", name=f"xt_{b}_{c}")
                        src = x[b][:, :, k0 : k0 + kcc, :]
                    else:
                        xt = xps[kcc].tile([P, kcc, w0], f8,
                                           tag=f"xt{kcc}", name=f"xt_{b}_{c}")
                        src = x[b].rearrange("(p k) e -> p k e", p=P)[
                            :, k0 : k0 + kcc, :
                        ]
                    loads.append((b, c, kcc, src, xt))
                    k0 += kcc
            order = sorted(range(len(loads)),
                           key=lambda i: (loads[i][0], loads[i][1]))
            xts = {}
            load_insts = {}
            for rank, i in enumerate(order):
                b, c, kcc, src, xt = loads[i]
                eng = nc.sync if rank % 2 == 0 else nc.scalar
                inst = eng.dma_start(out=xt[...], in_=src)
                xts[b, c] = xt
                load_insts[b, c] = inst
            # Throttle the bulk loads (batches 2-3) behind earlier chunk
            # completions: a chunk's completion semaphore fires only when
            # every SDMA engine drains its share, and the engines round-
            # robin across ALL queues with pending work - so a chunk's
            # completion time tracks the TOTAL bytes queued concurrently.
            # Keeping the early tranche to batches 0-1 makes their
            # semaphores fire right after their bytes land (the PE's HAM
            # warm-up window must not see a >1us starve), while batches
            # 2-3 stream in behind them well before the PE needs them.
            gates = {(2, 0): (0, 3), (2, 1): (1, 0),
                     (3, 0): (1, 1), (3, 1): (2, 0)}
            for (b, c), (gb, gc) in gates.items():
                if (b, c) in load_insts and (gb, gc) in load_insts:
                    add_dep_helper(load_insts[b, c].ins,
                                   load_insts[gb, gc].ins, sync=True,
                                   reason="throttle bulk load")



            def emit_kloop(b):
                ps0, ps1 = pss[b]
                nk = KP if mode in ("swi", "dr") else KT
                def mm(block, xt, k, **ss):
                    if mode == "swi":
                        pm = mybir.MatmulPerfMode.DoubleRowSwInterleave
                        mov = xt[:, k, :].rearrange(
                            "p (c two) -> p two c", two=2
                        )
                        if block == 0:
                            nc.tensor.matmul(
                                ps0[:, :], xt[:, k, 0 : 2 * P], mov[:, :, :],
                                perf_mode=pm, **ss,
                            )
                        else:
                            nc.tensor.matmul(
                                ps1[:, :], xt[:, k, 2 * P : 2 * d],
                                mov[:, :, P : d + 1], perf_mode=pm, **ss,
                            )
                    elif mode == "dr":
                        pm = mybir.MatmulPerfMode.DoubleRow
                        if block == 0:
                            nc.tensor.matmul(
                                ps0[:, :], xt[:, :, k, 0:P], xt[:, :, k, :],
                                perf_mode=pm, **ss,
                            )
                        else:
                            nc.tensor.matmul(
                                ps1[:, :], xt[:, :, k, P:d],
                                xt[:, :, k, P : d + 1], perf_mode=pm, **ss,
                            )
                    else:
                        if block == 0:
                            nc.tensor.matmul(
                                ps0[:, :], xt[:, k, 0:P], xt[:, k, :], **ss
                            )
                        else:
                            nc.tensor.matmul(
                                ps1[:, :], xt[:, k, P:d], xt[:, k, P : d + 1],
                                **ss,
                            )

                kk = 0
                for c, kcc in enumerate(_chunk_split(mode, b)):
                    xt = xts[b, c]
                    for k in range(kcc):
                        ss = dict(start=(kk == 0), stop=(kk == nk - 1))
                        mm(0, xt, k, **ss)
                        mm(1, xt, k, **ss)
                        kk += 1
                return ps0, ps1

            def emit_epilogue(b, last=False):
                ps0, ps1 = pss[b]
                ot = op.tile([P, w0 + w1], f16, tag="ot", name=f"ot_{b}")
                # Stage the two PSUM blocks on DIFFERENT engines so the
                # last batch's exposed tail is max(copy) not sum(copy).
                nc.vector.tensor_copy(ot[:, 0:w0], ps0[:, :])
                nc.scalar.copy(ot[:, w0 : w0 + w1], ps1[:, :])
                # Out-DMAs on the HWDGE rings: all input-load descriptors
                # are generated long before these queue, so their sem waits
                # stall idle rings; HWDGE's ~0.6us fixed cost beats SWDGE's
                # ~2us on the exposed last-batch tail. Mid-stream batches
                # use one trigger (descriptor gen is ~0.6us each; surplus
                # producer waits get hoisted into Drains by the JSON
                # post-pass). The LAST batch splits across both rings so
                # the two descriptor gens and HBM write receipts overlap.
                if last:
                    nc.sync.dma_start(out=outs[b][:, 0:w0], in_=ot[:, 0:w0])
                    nc.scalar.dma_start(
                        out=outs[b][:, w0 : w0 + w1], in_=ot[:, w0 : w0 + w1]
                    )
                else:
                    nc.sync.dma_start(out=outs[b], in_=ot[:, :])

            # One-batch software pipeline: epilogue(b) is emitted after
            # kloop(b+1) so the PE stream never stalls on the epilogue.
            for b in range(bpc):
                emit_kloop(b)
                if b > 0:
                    emit_epilogue(b - 1)
            emit_epilogue(bpc - 1, last=True)

    _install_drain_split(nc)
    return nc


def _split_drain_waits(bir, max_waits=1):
    """Cap sem waits per instruction at `max_waits` (the HW sync-wait table
    allows one wait per instruction). Drains with more waits split into a
    chain of single-wait Drains; DMACopy triggers with more waits get the
    surplus hoisted into inserted single-wait Drains on the same engine
    queue (FIFO order makes this equivalent: the queue stalls either way)."""
    for fn in bir["functions"]:
        for blk in fn["blocks"]:
            out = []
            changed = False
            for inst in blk["instructions"]:
                waits = (inst.get("sync_info") or {}).get("on_wait") or []
                if len(waits) > max_waits:
                    changed = True
                    for wi in range(0, len(waits) - max_waits):
                        out.append({
                            "debug": inst.get("debug"),
                            "engine": inst["engine"],
                            "ins": [],
                            "name": f"{inst['name']}_w{wi}",
                            "opcode": "Drain",
                            "outs": [],
                            "sync_info": {
                                "on_wait": [waits[wi]],
                                "on_update": [],
                            },
                        })
                    inst = {
                        **inst,
                        "sync_info": {
                            **inst["sync_info"],
                            "on_wait": waits[len(waits) - max_waits :],
                        },
                    }
                out.append(inst)
            if changed:
                blk["instructions"] = out
    return bir


def _install_drain_split(nc):
    import orjson

    raw = nc.to_json_bytes

    def patched():
        return orjson.dumps(_split_drain_waits(orjson.loads(raw())))

    nc.to_json_bytes = patched


_NC_CACHE = {}


def _get_nc(mode=MODE):
    key = (mode, BPC, N, D)
    if key not in _NC_CACHE:
        _NC_CACHE[key] = build_nc(mode)
    return _NC_CACHE[key]


def pack_input(feats, mode=MODE):
    """[cores, bpc, n, d] fp32 -> per-core fp8 device layout + ones col."""
    _, np8 = _F8[mode]
    nc_, bpc, n, d = feats.shape
    xa = np.empty((nc_, bpc, n, d + 1), dtype=np8)
    xa[..., :d] = feats.astype(np8)
    xa[..., d] = 1.0
    if mode == "swi":
        # [p, j, i, c] -> interleave pair rows element-wise: [p, j, c*2+i]
        return (
            xa.reshape(nc_, bpc, P, KP, 2, d + 1)
            .transpose(0, 1, 2, 3, 5, 4)
            .reshape(nc_, bpc, P, KP, 2 * (d + 1))
            .copy()
        )
    if mode == "dr":
        # [p, i, j, c]: row p*32 + i*16 + j
        return xa.reshape(nc_, bpc, P, 2, KP, d + 1).copy()
    return xa


def stats_from_raw(outs_blocks, mode=MODE, n=N, d=D):
    """Device outs [bz, 128, 386] (packed, see build_nc) -> f64 stats."""
    bz = outs_blocks.shape[0]
    h = d // 2
    o = outs_blocks.astype(np.float64)
    if mode == "swi":
        # SwInterleave weight columns are HW-reversed: S-row m sits in
        # output partition (127 - m) of each block.
        o = o[:, ::-1, :]
    s = np.empty((bz, d, d))
    s[:, :h, :] = o[:, :, 0:d]
    s[:, h:, h:] = o[:, :, d + 1 : d + 1 + h]
    s[:, h:, :h] = np.swapaxes(o[:, :, h:d], 1, 2)  # symmetry mirror
    colsum = np.concatenate([o[:, :, d], o[:, :, d + 1 + h]], axis=1)
    m = colsum / n
    covs = (s - colsum[:, :, None] * m[:, None, :]) / (n - 1)
    return m, covs


def coral_from_stats(means, covs, domains, d=D):
    """Masked pairwise CORAL reduction from per-batch stats (float64)."""
    bz = means.shape[0]
    m = means.astype(np.float64)
    ms = (m * m).sum(1)
    md = (ms[:, None] + ms[None, :] - 2.0 * (m @ m.T)) / d
    v = covs.astype(np.float64).reshape(bz, -1)
    cs = (v * v).sum(1)
    g = v @ v.T
    cd = (cs[:, None] + cs[None, :] - 2.0 * g) / (d * d)
    upper = np.triu(np.ones((bz, bz), dtype=bool), k=1)
    mask = upper & (np.asarray(domains)[:, None] != np.asarray(domains)[None, :])
    loss = np.where(mask, md + cd, 0.0).sum()
    num = int(mask.sum())
    if num > 1:
        loss = loss / num
    return np.float32(loss)


def kernel(features, domains, _trace=False, _mode=None):
    from concourse import bass_utils

    mode = _mode or MODE
    feats = np.asarray(features)
    assert feats.shape == (BZ, N, D)
    xpk = pack_input(
        np.asarray(feats, dtype=np.float32).reshape(NCORES, BPC, N, D), mode
    )
    nc = _get_nc(mode)
    in_maps = [{"x": xpk[c]} for c in range(NCORES)]
    res = bass_utils.run_bass_kernel_spmd(
        nc, in_maps, core_ids=list(range(NCORES)), trace=_trace
    )
    blocks = np.concatenate([r["outs"] for r in res.results], axis=0)
    means, covs = stats_from_raw(blocks, mode)
    out = coral_from_stats(means, covs, domains)
    if _trace:
        return out, res
    return out
